# revision 8
# baseline (speedup 1.0000x reference)
"""Trainium2 Bass kernel for nn_GTShapelet (GIN stack + CLS-query MHA).

Self-contained: builds the Bass/Tile program, shards inputs across 8
NeuronCores (data-parallel over destination-node ranges; graphs 4c..4c+3
on core c), runs via run_bass_kernel_spmd, and reassembles the full
[32, 128] output.

Key algorithmic points:
  - Only y[:, -1, :] (the CLS token) is returned by the reference, so the
    attention needs just one query per graph, and that query is
    h-independent (cls_embedding is shared): scores/softmax collapse to a
    [4, 1025] problem per graph.
  - Layer 1 folds embed_table @ W1 into a 1024x256 table T1 so the whole
    layer is gathers from T1: gelu(T1[nids] + sum_e ew*T1[nids[src]] + b1).
  - segment_sum is computed per 64-destination chunk as a sum of PE
    matmuls S_b^T.T @ X_b where X_b are 128 gathered source rows and
    S_b^T[e, d] = ew[e] * (dst_local[e] == d), built on DVE from shipped
    dst_local/ew vectors.
  - Between GIN layers the 8 cores exchange their [4096, 256] bf16 h
    shards with an AllGather so the next layer's gathers see all nodes.
"""

import sys

if "/opt/trn_rl_repo" not in sys.path:
    sys.path.insert(0, "/opt/trn_rl_repo")

import numpy as np
import ml_dtypes  # noqa: F401  (np 'bfloat16' dtype registration)

# ---- problem constants (hardcoded per spec) ----
B, N, E, D = 32, 1024, 524288, 128
H, HD = 4, 32
F2 = 2 * D  # 256
NCORES = 8
NPC = B * N // NCORES          # 4096 nodes per core
GPC = B // NCORES              # 4 graphs per core
CHW = 64                       # dst-chunk width (nodes)
NCH = NPC // CHW               # 64 chunks per core
NB = 10                        # edge batches (x128) per chunk (capacity 1280)
CHCAP = NB * 128               # 1280 edge slots per chunk
CAP = NCH * CHCAP              # 81920 slots per core
SEG = 8                        # chunks per S-build segment
BF16 = np.dtype('bfloat16')

_prog_cache = {}


def _build_program():
    if 'nc' in _prog_cache:
        return _prog_cache['nc']
    import concourse.bacc as bacc
    import concourse.tile as tile
    import concourse.mybir as mybir
    from concourse.library_config import mlp

    dt = mybir.dt
    AF = mybir.ActivationFunctionType
    OP = mybir.AluOpType

    nc = bacc.Bacc("TRN2", target_bir_lowering=False, debug=False,
                   num_devices=NCORES)

    def din(name, shape, dtype):
        return nc.dram_tensor(name, shape, dtype, kind="ExternalInput")

    t1 = din("t1", [N, F2], dt.bfloat16)
    h0w1own = din("h0w1own", [NPC, F2], dt.bfloat16)
    idx0 = din("idx0", [128, CAP // 16], dt.int16)
    idx12 = din("idx12", [128, CAP // 16], dt.int16)
    cnts = din("cnts", [1, 2 * NCH], dt.int32)
    dstl = din("dstl", [128, NCH * NB], dt.bfloat16)
    eww = din("eww", [128, NCH * NB], dt.bfloat16)
    iota64 = din("iota64", [128, CHW], dt.bfloat16)
    i64 = din("i64", [128, 64], dt.bfloat16)
    i128 = din("i128", [128, 128], dt.bfloat16)
    i128f = din("i128f", [128, 128], dt.float32)
    i4 = din("i4", [4, 4], dt.bfloat16)
    w2t = din("w2t", [128, 4 * 128], dt.bfloat16)
    b2 = din("b2", [128, 2], dt.float32)
    w3t = din("w3t", [128, 2 * 128], dt.bfloat16)
    b3 = din("b3", [128, 1], dt.float32)
    b1 = din("b1", [128, 2], dt.float32)
    wk = din("wk", [128, 128], dt.bfloat16)
    bk = din("bk", [128, 1], dt.float32)
    wv = din("wv", [128, 128], dt.bfloat16)
    bv = din("bv", [128, 1], dt.float32)
    qblk = din("qblk", [128, 4], dt.bfloat16)
    vc4 = din("vc4", [4, 128], dt.bfloat16)
    e4 = din("e4", [4, 4], dt.bfloat16)
    msel = din("msel", [128, 4], dt.float32)
    r4 = din("r4", [4, 128], dt.float32)
    ecls = din("ecls", [4, 1], dt.float32)
    eps = din("eps", [4, 1], dt.float32)
    ynb = din("ynb", [128, 1], dt.float32)
    wo = din("wo", [128, 128], dt.bfloat16)
    lng = din("lng", [4, 128], dt.float32)
    lnb = din("lnb", [4, 128], dt.float32)
    y_out = nc.dram_tensor("y", [GPC, D], dt.float32, kind="ExternalOutput")

    with tile.TileContext(nc) as tc:
        nc.gpsimd.load_library(mlp)
        with tc.tile_pool(name="const", bufs=1) as cp, \
             tc.tile_pool(name="res", bufs=1) as rp, \
             tc.tile_pool(name="work", bufs=1) as wp, \
             tc.tile_pool(name="dram", bufs=2, space="DRAM") as dram:

            def cload(ap, shape, dtype):
                t = cp.tile(shape, dtype, name=f"c_{ap.name}")
                nc.sync.dma_start(out=t[:], in_=ap[:])
                return t

            # t1 stays in DRAM (gather source) -- no SBUF copy needed.
            idx0_t = cload(idx0, [128, CAP // 16], dt.int16)
            idx12_t = cload(idx12, [128, CAP // 16], dt.int16)
            cnts_t = cload(cnts, [1, 2 * NCH], dt.int32)
            dstl_t = cload(dstl, [128, NCH * NB], dt.bfloat16)
            eww_t = cload(eww, [128, NCH * NB], dt.bfloat16)
            iota_t = cload(iota64, [128, CHW], dt.bfloat16)
            i64_t = cload(i64, [128, 64], dt.bfloat16)
            i128_t = cload(i128, [128, 128], dt.bfloat16)
            i128f_t = cload(i128f, [128, 128], dt.float32)
            i4_t = cload(i4, [4, 4], dt.bfloat16)
            w2t_t = cload(w2t, [128, 4 * 128], dt.bfloat16)
            b2_t = cload(b2, [128, 2], dt.float32)
            w3t_t = cload(w3t, [128, 2 * 128], dt.bfloat16)
            b3_t = cload(b3, [128, 1], dt.float32)
            b1_t = cload(b1, [128, 2], dt.float32)
            wk_t = cload(wk, [128, 128], dt.bfloat16)
            bk_t = cload(bk, [128, 1], dt.float32)
            wv_t = cload(wv, [128, 128], dt.bfloat16)
            bv_t = cload(bv, [128, 1], dt.float32)
            qblk_t = cload(qblk, [128, 4], dt.bfloat16)
            vc4_t = cload(vc4, [4, 128], dt.bfloat16)
            e4_t = cload(e4, [4, 4], dt.bfloat16)
            msel_t = cload(msel, [128, 4], dt.float32)
            r4_t = cload(r4, [4, 128], dt.float32)
            ecls_t = cload(ecls, [4, 1], dt.float32)
            eps_t = cload(eps, [4, 1], dt.float32)
            ynb_t = cload(ynb, [128, 1], dt.float32)
            wo_t = cload(wo, [128, 128], dt.bfloat16)
            lng_t = cload(lng, [4, 128], dt.float32)
            lnb_t = cload(lnb, [4, 128], dt.float32)

            # persistent per-layer state (double-buffered by hand)
            hown = [rp.tile([128, NPC // 128, F2], dt.bfloat16, tag=f"hown{i}",
                            name=f"hown{i}") for i in range(2)]
            hT = [rp.tile([128, 2, NPC], dt.bfloat16, tag=f"hT{i}",
                          name=f"hT{i}") for i in range(2)]
            rhsT = rp.tile([128, 2, NPC], dt.bfloat16, tag="rhsT")
            # gather X buffers: fixed rotation, memset once for pad safety
            xbufs = [wp.tile([128, NB, F2], dt.bfloat16, tag=f"x{i}",
                             name=f"xbuf{i}") for i in range(3)]
            for xb in xbufs:
                nc.vector.memset(xb[:], 0)
            sseg = [wp.tile([128, SEG * NB * CHW], dt.bfloat16, tag=f"sseg{i}",
                            name=f"sseg{i}") for i in range(2)]

            # layer 0's "own" rows stream in from DRAM
            nc.sync.dma_start(
                out=hown[0][:],
                in_=h0w1own.rearrange("(t p) f -> p t f", p=128))

            gather_srcs = [t1]
            with tc.tile_pool(name="gin_ps", bufs=1, space="PSUM") as pp, \
                 tc.tile_pool(name="gin_sb", bufs=1) as gp:
                for l in range(3):
                    idx_t = idx0_t if l == 0 else idx12_t
                    gsrc = gather_srcs[l]
                    own = hown[l % 2]
                    hT_cur = hT[l % 2]
                    for s in range(NCH // SEG):
                        # build selection matrices for chunks [s*SEG, (s+1)*SEG)
                        st = sseg[s % 2]
                        bb0 = s * SEG * NB
                        nbat = SEG * NB
                        sv = st[:].rearrange("p (b d) -> p b d", d=CHW)
                        nc.vector.tensor_tensor(
                            out=sv,
                            in0=dstl_t[:, bb0:bb0 + nbat].to_broadcast(
                                [128, nbat, CHW]),
                            in1=iota_t[:].unsqueeze(1).broadcast_to(
                                [128, nbat, CHW]),
                            op=OP.is_equal)
                        nc.vector.tensor_tensor(
                            out=sv, in0=sv,
                            in1=eww_t[:, bb0:bb0 + nbat].to_broadcast(
                                [128, nbat, CHW]),
                            op=OP.mult)
                        for kk in range(SEG):
                            k = s * SEG + kk
                            xb = xbufs[k % 3]
                            ib = k * (CHCAP // 16)
                            crega = nc.gpsimd.value_load(cnts_t[0:1, 2 * k:2 * k + 1])
                            nc.gpsimd.dma_gather(
                                xb[:, 0:8, :], gsrc[:], idx_t[:, ib:ib + 64],
                                1024, crega, F2)
                            cregb = nc.gpsimd.value_load(cnts_t[0:1, 2 * k + 1:2 * k + 2])
                            nc.gpsimd.dma_gather(
                                xb[:, 8:10, :], gsrc[:], idx_t[:, ib + 64:ib + 80],
                                256, cregb, F2)
                            ps = pp.tile([64, F2], dt.float32, tag="seg", bufs=2)
                            for bq in range(NB):
                                c0 = (kk * NB + bq) * CHW
                                nc.tensor.matmul(
                                    out=ps[:], lhsT=st[:, c0:c0 + CHW],
                                    rhs=xb[:, bq, :],
                                    start=(bq == 0), stop=False)
                            ochunk = own[(k % 2) * 64:(k % 2) * 64 + 64, k // 2, :]
                            nc.tensor.matmul(out=ps[:],
                                             lhsT=i64_t[(k % 2) * 64:(k % 2) * 64 + 64, :],
                                             rhs=ochunk, start=False, stop=True)
                            msb = gp.tile([64, F2], dt.bfloat16, tag="msb", bufs=3)
                            nc.scalar.activation(msb[:], ps[:], AF.Copy)
                            for j in range(2):
                                tp = pp.tile([128, 64], dt.bfloat16, tag="tp", bufs=2)
                                nc.tensor.transpose(
                                    tp[:], msb[:, j * 128:(j + 1) * 128], i64_t[0:64, :])
                                dst_col = slice(k * 64, (k + 1) * 64)
                                if l == 0:
                                    nc.scalar.activation(
                                        hT_cur[:, j, dst_col], tp[:], AF.Gelu,
                                        bias=b1_t[:, j:j + 1])
                                else:
                                    nc.vector.tensor_copy(
                                        out=rhsT[:, j, dst_col], in_=tp[:])
                    if l > 0:
                        # node matmul with W{l+1} + gelu
                        wt, bt = (w2t_t, b2_t) if l == 1 else (w3t_t, b3_t)
                        fouth = 2 if l == 1 else 1
                        for jo in range(fouth):
                            for m in range(NPC // 512):
                                ps2 = pp.tile([128, 512], dt.float32, tag="nm", bufs=2)
                                for ji in range(2):
                                    if l == 1:
                                        wslice = wt[:, (2 * ji + jo) * 128:(2 * ji + jo + 1) * 128]
                                    else:
                                        wslice = wt[:, ji * 128:(ji + 1) * 128]
                                    nc.tensor.matmul(
                                        out=ps2[:], lhsT=wslice,
                                        rhs=rhsT[:, ji, m * 512:(m + 1) * 512],
                                        start=(ji == 0), stop=(ji == 1))
                                nc.scalar.activation(
                                    hT_cur[:, jo, m * 512:(m + 1) * 512],
                                    ps2[:], AF.Gelu, bias=bt[:, jo:jo + 1])
                    if l < 2:
                        # transpose hT -> node-major, then AllGather
                        hon = hown[(l + 1) % 2]
                        for t in range(NPC // 128):
                            for j in range(2):
                                tp2 = pp.tile([128, 128], dt.bfloat16, tag="tp2", bufs=2)
                                nc.tensor.transpose(
                                    tp2[:], hT_cur[:, j, t * 128:(t + 1) * 128],
                                    i128_t[:])
                                nc.vector.tensor_copy(
                                    out=hon[:, t, j * 128:(j + 1) * 128],
                                    in_=tp2[:])
                        agin = dram.tile([NPC, F2], dt.bfloat16, tag="agin")
                        agout = dram.tile([B * N, F2], dt.bfloat16, tag="agout")
                        nc.sync.dma_start(
                            out=agin.rearrange("(t p) f -> p t f", p=128),
                            in_=hon[:])
                        nc.gpsimd.collective_compute(
                            "AllGather", OP.bypass,
                            replica_groups=[list(range(NCORES))],
                            ins=[agin.opt()], outs=[agout.opt()])
                        gather_srcs.append(agout)

            # ---------------- attention + layernorm ----------------
            h3T = hT[2 % 2]  # [128, 2, NPC]; only [:, 0, :] is meaningful
            with tc.tile_pool(name="att_ps", bufs=1, space="PSUM") as ap_, \
                 tc.tile_pool(name="att_sb", bufs=1) as asb:
                kT = asb.tile([128, NPC], dt.bfloat16, tag="kT")
                vnm = asb.tile([128, NPC // 128, 128], dt.bfloat16, tag="vnm")
                for m in range(NPC // 512):
                    psk = ap_.tile([128, 512], dt.float32, tag="pbig", bufs=2)
                    nc.tensor.matmul(out=psk[:], lhsT=wk_t[:],
                                     rhs=h3T[:, 0, m * 512:(m + 1) * 512])
                    nc.vector.tensor_scalar(
                        out=kT[:, m * 512:(m + 1) * 512], in0=psk[:],
                        scalar1=bk_t[:], scalar2=None, op0=OP.add)
                for t in range(NPC // 128):
                    psv = ap_.tile([128, 128], dt.float32, tag="pbig", bufs=2)
                    nc.tensor.matmul(out=psv[:],
                                     lhsT=h3T[:, 0, t * 128:(t + 1) * 128],
                                     rhs=wv_t[:])
                    nc.vector.tensor_copy(out=vnm[:, t, :], in_=psv[:])
                ctx_all = asb.tile([128, 4], dt.bfloat16, tag="ctx_all")
                for g in range(GPC):
                    ssc = ap_.tile([4, 1024], dt.float32, tag="pbig", bufs=2)
                    for hh in range(2):
                        nc.tensor.matmul(
                            out=ssc[:, hh * 512:(hh + 1) * 512], lhsT=qblk_t[:],
                            rhs=kT[:, g * 1024 + hh * 512: g * 1024 + (hh + 1) * 512])
                    expt = asb.tile([4, 1024], dt.bfloat16, tag="expt")
                    sums = asb.tile([4, 1], dt.float32, tag="sums")
                    nc.scalar.activation(expt[:], ssc[:], AF.Exp,
                                         accum_out=sums[:])
                    nc.vector.tensor_add(out=sums[:], in0=sums[:], in1=ecls_t[:])
                    psr = ap_.tile([128, 1], dt.float32, tag="ptiny", bufs=2)
                    nc.tensor.matmul(out=psr[:], lhsT=r4_t[:], rhs=sums[:])
                    rbc = asb.tile([128, 1], dt.float32, tag="rbc")
                    nc.vector.reciprocal(rbc[:], psr[:])
                    psctx = ap_.tile([128, 4], dt.float32, tag="psctx", bufs=1)
                    for t in range(8):
                        pst = ap_.tile([128, 4], dt.bfloat16, tag="ptiny", bufs=2)
                        nc.tensor.transpose(
                            pst[:], expt[:, t * 128:(t + 1) * 128], i4_t[:])
                        ets = asb.tile([128, 4], dt.bfloat16, tag="ets")
                        nc.vector.tensor_copy(out=ets[:], in_=pst[:])
                        nc.tensor.matmul(out=psctx[:],
                                         lhsT=vnm[:, g * 8 + t, :], rhs=ets[:],
                                         start=(t == 0), stop=False)
                    nc.tensor.matmul(out=psctx[:], lhsT=vc4_t[:], rhs=e4_t[:],
                                     start=False, stop=True)
                    tmp4 = asb.tile([128, 4], dt.float32, tag="tmp4")
                    nc.vector.tensor_tensor(out=tmp4[:], in0=psctx[:],
                                            in1=msel_t[:], op=OP.mult)
                    ctxv = asb.tile([128, 1], dt.float32, tag="ctxv")
                    nc.vector.reduce_sum(out=ctxv[:], in_=tmp4[:],
                                         axis=mybir.AxisListType.X)
                    nc.vector.tensor_scalar(out=ctxv[:], in0=ctxv[:],
                                            scalar1=rbc[:], scalar2=bv_t[:],
                                            op0=OP.mult, op1=OP.add)
                    nc.vector.tensor_copy(out=ctx_all[:, g:g + 1], in_=ctxv[:])
                psao = ap_.tile([128, 4], dt.float32, tag="ptiny", bufs=2)
                nc.tensor.matmul(out=psao[:], lhsT=wo_t[:], rhs=ctx_all[:])
                ysb = asb.tile([128, 4], dt.float32, tag="ysb")
                nc.vector.tensor_scalar(out=ysb[:], in0=psao[:],
                                        scalar1=ynb_t[:], scalar2=None,
                                        op0=OP.add)
                psy = ap_.tile([4, 128], dt.float32, tag="ptiny", bufs=2)
                nc.tensor.matmul(out=psy[:], lhsT=ysb[:], rhs=i128f_t[:],
                                 is_transpose=True)
                yt = asb.tile([4, 128], dt.float32, tag="yt")
                nc.vector.tensor_copy(out=yt[:], in_=psy[:])
                mn = asb.tile([4, 1], dt.float32, tag="mn")
                nc.vector.reduce_sum(out=mn[:], in_=yt[:],
                                     axis=mybir.AxisListType.X)
                nc.vector.tensor_scalar(out=mn[:], in0=mn[:],
                                        scalar1=1.0 / D, scalar2=None,
                                        op0=OP.mult)
                xc = asb.tile([4, 128], dt.float32, tag="xc")
                nc.vector.tensor_scalar(out=xc[:], in0=yt[:], scalar1=mn[:],
                                        scalar2=None, op0=OP.subtract)
                sq = asb.tile([4, 128], dt.float32, tag="sq")
                ss = asb.tile([4, 1], dt.float32, tag="ss")
                nc.scalar.activation(sq[:], xc[:], AF.Square, accum_out=ss[:])
                sd = asb.tile([4, 1], dt.float32, tag="sd")
                nc.scalar.activation(sd[:], ss[:], AF.Sqrt, bias=eps_t[:],
                                     scale=1.0 / D)
                rr = asb.tile([4, 1], dt.float32, tag="rr")
                nc.vector.reciprocal(rr[:], sd[:])
                yn = asb.tile([4, 128], dt.float32, tag="yn")
                nc.vector.tensor_scalar(out=yn[:], in0=xc[:], scalar1=rr[:],
                                        scalar2=None, op0=OP.mult)
                nc.vector.tensor_tensor(out=yn[:], in0=yn[:], in1=lng_t[:],
                                        op=OP.mult)
                nc.vector.tensor_tensor(out=yn[:], in0=yn[:], in1=lnb_t[:],
                                        op=OP.add)
                nc.sync.dma_start(out=y_out[:], in_=yn[:])

    nc.compile()
    _prog_cache['nc'] = nc
    return nc


def _wrap16(arr):
    """slot i -> [i % 16, i // 16], replicated into partitions 16..31.

    CoreSim's gather ucode reads partitions 0..15; the deployed HW ucode
    reads 16..31 -- fill both so either path sees the indices.
    """
    n = arr.shape[0]
    out = np.zeros((128, n // 16), np.int16)
    w = arr.reshape(n // 16, 16).T.astype(np.int16)
    out[0:16] = w
    out[16:32] = w
    return out


def _host_prep(inputs):
    node_ids = np.asarray(inputs["node_ids"]).astype(np.int64)
    src = np.asarray(inputs["src"]).astype(np.int64)
    dst = np.asarray(inputs["dst"]).astype(np.int64)
    pad_mask = np.asarray(inputs["pad_mask"])
    ew = np.asarray(inputs["edge_weight"]).astype(np.float64)
    embed = np.asarray(inputs["embed_table"]).astype(np.float64)
    W1 = np.asarray(inputs["W1"]).astype(np.float64)
    b1 = np.asarray(inputs["b1"]).astype(np.float32)
    W2 = np.asarray(inputs["W2"]).astype(np.float32)
    b2 = np.asarray(inputs["b2"]).astype(np.float32)
    W3 = np.asarray(inputs["W3"]).astype(np.float32)
    b3 = np.asarray(inputs["b3"]).astype(np.float32)
    ipw = np.asarray(inputs["in_proj_w"]).astype(np.float64)
    ipb = np.asarray(inputs["in_proj_b"]).astype(np.float64)
    ow = np.asarray(inputs["out_w"]).astype(np.float32)
    ob = np.asarray(inputs["out_b"]).astype(np.float32)
    cls = np.asarray(inputs["cls_embedding"]).astype(np.float64).reshape(D)
    ln_g = np.asarray(inputs["ln_g"]).astype(np.float32)
    ln_b = np.asarray(inputs["ln_b"]).astype(np.float32)

    assert not pad_mask.any(), "kernel compiled for all-False pad_mask"

    # ---- shared (replicated) constants ----
    T1 = (embed @ W1).astype(BF16)                       # [1024, 256]
    Wq, Wk, Wv = ipw[:, :D], ipw[:, D:2 * D], ipw[:, 2 * D:]
    bq, bk_, bv_ = ipb[:D], ipb[D:2 * D], ipb[2 * D:]
    q_cls = (cls @ Wq + bq) / np.sqrt(HD)                # [128]
    k_cls = cls @ Wk + bk_
    v_cls = cls @ Wv + bv_
    s_cls = np.array([q_cls[h * HD:(h + 1) * HD] @ k_cls[h * HD:(h + 1) * HD]
                      for h in range(H)])                # [4]
    e_cls = np.exp(s_cls)
    qblk = np.zeros((128, 4), np.float32)
    for h in range(H):
        qblk[h * HD:(h + 1) * HD, h] = q_cls[h * HD:(h + 1) * HD]
    vc4 = np.zeros((4, 128), np.float32)
    for h in range(H):
        vc4[h, h * HD:(h + 1) * HD] = v_cls[h * HD:(h + 1) * HD]
    e4 = np.diag(e_cls).astype(np.float32)
    msel = np.zeros((128, 4), np.float32)
    for h in range(H):
        msel[h * HD:(h + 1) * HD, h] = 1.0
    r4 = np.zeros((4, 128), np.float32)
    for h in range(H):
        r4[h, h * HD:(h + 1) * HD] = 1.0
    w2tiles = np.concatenate(
        [W2[ji * 128:(ji + 1) * 128, jo * 128:(jo + 1) * 128]
         for ji in range(2) for jo in range(2)], axis=1)  # [128, 512]
    w3tiles = np.concatenate(
        [W3[ji * 128:(ji + 1) * 128, :] for ji in range(2)], axis=1)
    shared = {
        "t1": T1,
        "iota64": np.tile(np.arange(CHW, dtype=np.float32), (128, 1)).astype(BF16),
        "i64": np.vstack([np.eye(64, dtype=np.float32)] * 2).astype(BF16),
        "i128": np.eye(128, dtype=np.float32).astype(BF16),
        "i128f": np.eye(128, dtype=np.float32),
        "i4": np.eye(4, dtype=np.float32).astype(BF16),
        "w2t": w2tiles.astype(BF16),
        "b2": b2.reshape(2, 128).T.copy(),
        "w3t": w3tiles.astype(BF16),
        "b3": b3.reshape(1, 128).T.copy(),
        "b1": b1.astype(np.float32).reshape(2, 128).T.copy(),
        "wk": Wk.astype(BF16),
        "bk": bk_.astype(np.float32).reshape(128, 1),
        "wv": Wv.astype(BF16),
        "bv": bv_.astype(np.float32).reshape(128, 1),
        "qblk": qblk.astype(BF16),
        "vc4": vc4.astype(BF16),
        "e4": e4.astype(BF16),
        "msel": msel,
        "r4": r4,
        "ecls": e_cls.astype(np.float32).reshape(4, 1),
        "eps": np.full((4, 1), 1e-5, np.float32),
        "ynb": (cls + ob).astype(np.float32).reshape(128, 1),
        "wo": ow.astype(BF16),
        "lng": np.tile(ln_g, (4, 1)),
        "lnb": np.tile(ln_b, (4, 1)),
    }

    # ---- per-core edge partitioning ----
    ew32 = ew.astype(np.float32)
    core_of = dst >> 12           # dst // 4096
    in_maps = []
    order_all = np.argsort(dst, kind='stable')
    dst_sorted = dst[order_all]
    core_starts = np.searchsorted(dst_sorted, np.arange(0, B * N + 1, NPC))
    chunk_starts = np.searchsorted(dst_sorted, np.arange(0, B * N + 1, CHW))
    for c in range(NCORES):
        lo, hi = core_starts[c], core_starts[c + 1]
        eidx = order_all[lo:hi]
        # slot arrays
        g_idx0 = np.zeros(CAP, np.int64)
        g_idx12 = np.full(CAP, -1, np.int64)
        g_idx0[:] = -1
        sl_dst = np.full(CAP, 100.0, np.float32)   # pad dst_local >= CHW
        sl_ew = np.zeros(CAP, np.float32)
        counts = np.zeros(2 * NCH, np.int32)
        base_chunk = c * NCH
        for k in range(NCH):
            a = chunk_starts[base_chunk + k] - lo
            bnd = chunk_starts[base_chunk + k + 1] - lo
            cnt = bnd - a
            assert cnt <= CHCAP, f"chunk overflow: {cnt} > {CHCAP}"
            e = eidx[a:bnd]
            s0 = k * CHCAP
            g_idx12[s0:s0 + cnt] = src[e]
            g_idx0[s0:s0 + cnt] = node_ids[src[e]]
            sl_dst[s0:s0 + cnt] = (dst[e] - (c * NPC + k * CHW)).astype(np.float32)
            sl_ew[s0:s0 + cnt] = ew32[e]
            if cnt == 0:
                g_idx12[s0] = 0
                g_idx0[s0] = 0
                cnt = 1
            counts[2 * k] = min(cnt, 1024)
            counts[2 * k + 1] = max(cnt - 1024, 1)
            if cnt <= 1024:
                g_idx12[s0 + 1024] = 0
                g_idx0[s0 + 1024] = 0
        nids_own = node_ids[c * NPC:(c + 1) * NPC]
        m = dict(shared)
        m.update({
            "h0w1own": T1.astype(np.float32)[nids_own].astype(BF16),
            "idx0": _wrap16(g_idx0),
            "idx12": _wrap16(g_idx12),
            "cnts": counts.reshape(1, 2 * NCH),
            "dstl": sl_dst.reshape(NCH * NB, 128).T.astype(BF16).copy(),
            "eww": sl_ew.reshape(NCH * NB, 128).T.astype(BF16).copy(),
        })
        in_maps.append(m)
    return in_maps


def kernel(**inputs):
    from concourse.bass_utils import run_bass_kernel_spmd
    nc = _build_program()
    in_maps = _host_prep(inputs)
    res = run_bass_kernel_spmd(nc, in_maps, core_ids=list(range(NCORES)))
    y = np.concatenate([res.results[c]["y"] for c in range(NCORES)], axis=0)
    return np.ascontiguousarray(y.astype(np.float32))


# revision 11
# speedup vs baseline: 1.2796x; 1.2796x over previous
"""Trainium2 Bass kernel for nn_GTShapelet (GIN stack + CLS-query MHA).

Self-contained: builds the Bass/Tile program, shards inputs across 8
NeuronCores (data-parallel over destination-node ranges; graphs 4c..4c+3
on core c), runs via run_bass_kernel_spmd, and reassembles the full
[32, 128] output.

Key algorithmic points:
  - Only y[:, -1, :] (the CLS token) is returned by the reference, so the
    attention needs just one query per graph, and that query is
    h-independent (cls_embedding is shared): scores/softmax collapse to a
    [4, 1025] problem per graph.
  - Layer 1 folds embed_table @ W1 into a 1024x256 table T1 so the whole
    layer is gathers from T1: gelu(T1[nids] + sum_e ew*T1[nids[src]] + b1).
  - segment_sum is computed per 64-destination chunk as a sum of PE
    matmuls S_b^T.T @ X_b where X_b are 128 gathered source rows and
    S_b^T[e, d] = ew[e] * (dst_local[e] == d), built on DVE from shipped
    dst_local/ew vectors.
  - Between GIN layers the 8 cores exchange their [4096, 256] bf16 h
    shards with an AllGather so the next layer's gathers see all nodes.
"""

import sys

if "/opt/trn_rl_repo" not in sys.path:
    sys.path.insert(0, "/opt/trn_rl_repo")

import numpy as np
import ml_dtypes  # noqa: F401  (np 'bfloat16' dtype registration)

# ---- problem constants (hardcoded per spec) ----
B, N, E, D = 32, 1024, 524288, 128
H, HD = 4, 32
F2 = 2 * D  # 256
NCORES = 8
NPC = B * N // NCORES          # 4096 nodes per core
GPC = B // NCORES              # 4 graphs per core
CHW = 64                       # dst-chunk width (nodes)
NCH = NPC // CHW               # 64 chunks per core
NB = 9                         # batches per chunk: 8 main + 1 tail
ACAP = 1024                    # main gather slots per chunk (one call)
TAIL = 128                     # tail slots per chunk
CHCAP = ACAP + TAIL            # 1152 edge capacity per chunk
SEG = 8                        # chunks per segment
NSEG = NCH // SEG              # 8 segments per core
SEGSLOTS = SEG * ACAP + SEG * TAIL   # 9216: [8x1024 main | 8x128 tails]
CAP = NSEG * SEGSLOTS          # 73728 slots per core
NBAT = SEG * NB                # 72 batches per segment (64 main + 8 tail)
BF16 = np.dtype('bfloat16')

_prog_cache = {}


def _build_program(variant="hw"):
    if variant in _prog_cache:
        return _prog_cache[variant]
    import concourse.bacc as bacc
    import concourse.tile as tile
    import concourse.mybir as mybir
    from concourse.library_config import mlp

    dt = mybir.dt
    AF = mybir.ActivationFunctionType
    OP = mybir.AluOpType

    nc = bacc.Bacc("TRN2", target_bir_lowering=False, debug=False,
                   num_devices=(1 if variant == "sim1" else NCORES))

    def din(name, shape, dtype):
        return nc.dram_tensor(name, shape, dtype, kind="ExternalInput")

    t1 = din("t1", [N, F2], dt.bfloat16)
    h0w1own = din("h0w1own", [NPC, F2], dt.bfloat16)
    ct = din("ct", [128, N // 128, NPC], dt.bfloat16)
    idx12 = din("idx12", [128, CAP // 16], dt.int16)
    cnts = din("cnts", [1, NCH + NSEG], dt.int32)
    dstl = din("dstl", [128, NSEG * NBAT], dt.bfloat16)
    eww = din("eww", [128, NSEG * NBAT], dt.bfloat16)
    iota_bd = din("iota_bd", [128, CHW * NBAT], dt.bfloat16)
    i64 = din("i64", [128, 64], dt.bfloat16)
    i128 = din("i128", [128, 128], dt.bfloat16)
    i128f = din("i128f", [128, 128], dt.float32)
    i4 = din("i4", [4, 4], dt.bfloat16)
    w2t = din("w2t", [128, 4 * 128], dt.bfloat16)
    b2 = din("b2", [128, 2], dt.float32)
    w3t = din("w3t", [128, 2 * 128], dt.bfloat16)
    b3 = din("b3", [128, 1], dt.float32)
    b1 = din("b1", [128, 2], dt.float32)
    wk = din("wk", [128, 128], dt.bfloat16)
    bk = din("bk", [128, 1], dt.float32)
    wv = din("wv", [128, 128], dt.bfloat16)
    bv = din("bv", [128, 1], dt.float32)
    qblk = din("qblk", [128, 4], dt.bfloat16)
    vc4 = din("vc4", [4, 128], dt.bfloat16)
    e4 = din("e4", [4, 4], dt.bfloat16)
    msel = din("msel", [128, 4], dt.float32)
    r4 = din("r4", [4, 128], dt.float32)
    ecls = din("ecls", [4, 1], dt.float32)
    eps = din("eps", [4, 1], dt.float32)
    ynb = din("ynb", [128, 1], dt.float32)
    wo = din("wo", [128, 128], dt.bfloat16)
    lng = din("lng", [4, 128], dt.float32)
    lnb = din("lnb", [4, 128], dt.float32)
    y_out = nc.dram_tensor("y", [GPC, D], dt.float32, kind="ExternalOutput")

    with tile.TileContext(nc) as tc:
        nc.gpsimd.load_library(mlp)
        with tc.tile_pool(name="const", bufs=1) as cp, \
             tc.tile_pool(name="res", bufs=1) as rp, \
             tc.tile_pool(name="work", bufs=1) as wp, \
             tc.tile_pool(name="dram", bufs=2, space="DRAM") as dram:

            def cload(ap, shape, dtype):
                t = cp.tile(shape, dtype, name=f"c_{ap.name}")
                nc.sync.dma_start(out=t[:], in_=ap[:])
                return t

            # t1 stays in DRAM (gather source) -- no SBUF copy needed.
            idx12_t = cload(idx12, [128, CAP // 16], dt.int16)
            cnts_t = cload(cnts, [1, NCH + NSEG], dt.int32)
            dstl_t = cload(dstl, [128, NSEG * NBAT], dt.bfloat16)
            eww_t = cload(eww, [128, NSEG * NBAT], dt.bfloat16)
            iota_t = cload(iota_bd, [128, CHW * NBAT], dt.bfloat16)
            i64_t = cload(i64, [128, 64], dt.bfloat16)
            i128_t = cload(i128, [128, 128], dt.bfloat16)
            i128f_t = cload(i128f, [128, 128], dt.float32)
            i4_t = cload(i4, [4, 4], dt.bfloat16)
            w2t_t = cload(w2t, [128, 4 * 128], dt.bfloat16)
            b2_t = cload(b2, [128, 2], dt.float32)
            w3t_t = cload(w3t, [128, 2 * 128], dt.bfloat16)
            b3_t = cload(b3, [128, 1], dt.float32)
            b1_t = cload(b1, [128, 2], dt.float32)
            wk_t = cload(wk, [128, 128], dt.bfloat16)
            bk_t = cload(bk, [128, 1], dt.float32)
            wv_t = cload(wv, [128, 128], dt.bfloat16)
            bv_t = cload(bv, [128, 1], dt.float32)
            qblk_t = cload(qblk, [128, 4], dt.bfloat16)
            vc4_t = cload(vc4, [4, 128], dt.bfloat16)
            e4_t = cload(e4, [4, 4], dt.bfloat16)
            msel_t = cload(msel, [128, 4], dt.float32)
            r4_t = cload(r4, [4, 128], dt.float32)
            ecls_t = cload(ecls, [4, 1], dt.float32)
            eps_t = cload(eps, [4, 1], dt.float32)
            ynb_t = cload(ynb, [128, 1], dt.float32)
            wo_t = cload(wo, [128, 128], dt.bfloat16)
            lng_t = cload(lng, [4, 128], dt.float32)
            lnb_t = cload(lnb, [4, 128], dt.float32)

            # persistent per-layer state (double-buffered by hand)
            hown = [rp.tile([128, NPC // 128, F2], dt.bfloat16, tag=f"hown{i}",
                            name=f"hown{i}") for i in range(2)]
            hT = [rp.tile([128, 2, NPC], dt.bfloat16, tag=f"hT{i}",
                          name=f"hT{i}") for i in range(2)]
            rhsT = rp.tile([128, 2, NPC], dt.bfloat16, tag="rhsT")
            # gather X buffers: fixed rotation, memset once for pad safety
            xbufs = [wp.tile([128, 8, F2], dt.bfloat16, tag=f"x{i}",
                             name=f"xbuf{i}") for i in range(3)]
            tailbufs = [wp.tile([128, SEG, F2], dt.bfloat16, tag=f"tb{i}",
                                name=f"tailbuf{i}") for i in range(2)]
            for tb in tailbufs:
                nc.vector.memset(tb[:], 0)
            for xb in xbufs:
                nc.vector.memset(xb[:], 0)
            sseg = [wp.tile([128, NBAT * CHW], dt.bfloat16, tag=f"sseg{i}",
                            name=f"sseg{i}") for i in range(2)]

            t1sb = rp.tile([128, N // 128, F2], dt.bfloat16, name="t1sb")
            nc.sync.dma_start(out=t1sb[:],
                              in_=t1.rearrange("(kk p) f -> p kk f", p=128))
            ctbufs = [wp.tile([128, N // 128, SEG * CHW], dt.bfloat16,
                              tag=f"ctb{i}", name=f"ctb{i}") for i in range(2)]
            # layer 0's "own" rows stream in from DRAM
            nc.sync.dma_start(
                out=hown[0][:],
                in_=h0w1own.rearrange("(t p) f -> p t f", p=128))

            gather_srcs = [t1]
            with tc.tile_pool(name="gin_ps", bufs=1, space="PSUM") as pp, \
                 tc.tile_pool(name="gin_sb", bufs=1) as gp:
                idx_t = idx12_t
                for l in range(3):
                    gsrc = gather_srcs[l]
                    own = hown[l % 2]
                    hT_cur = hT[l % 2]
                    for s in range(NSEG):
                        nbat = NBAT
                        if l == 0:
                            ctb = ctbufs[s % 2]
                            nc.sync.dma_start(
                                out=ctb[:],
                                in_=ct[:, :, s * SEG * CHW:(s + 1) * SEG * CHW])
                        else:
                            # selection matrices for chunks [s*SEG, (s+1)*SEG)
                            # layout: element (d, b) at col d*nbat + b so the
                            # last AP dim is stride-1 (DVE 2x eligibility)
                            st = sseg[s % 2]
                            bb0 = s * NBAT
                            # one merged gather for all 8 chunks' tails
                            tb = tailbufs[s % 2]
                            tib = (s * SEGSLOTS + SEG * ACAP) // 16
                            cregt = nc.gpsimd.value_load(
                                cnts_t[0:1, NCH + s:NCH + s + 1])
                            nc.gpsimd.dma_gather(
                                tb[:], gsrc[:], idx_t[:, tib:tib + 64],
                                1024, cregt, F2)
                            sv = st[:].rearrange("p (d b) -> p d b", b=nbat)
                            nc.vector.tensor_tensor(
                                out=sv,
                                in0=dstl_t[:, bb0:bb0 + nbat].unsqueeze(1)
                                    .broadcast_to([128, CHW, nbat]),
                                in1=iota_t[:].rearrange(
                                    "p (d b) -> p d b", b=nbat),
                                op=OP.is_equal)
                            nc.vector.tensor_tensor(
                                out=sv, in0=sv,
                                in1=eww_t[:, bb0:bb0 + nbat].unsqueeze(1)
                                    .broadcast_to([128, CHW, nbat]),
                                op=OP.mult)
                        for kk in range(SEG):
                            k = s * SEG + kk
                            ps = pp.tile([64, F2], dt.float32, tag="seg", bufs=2)
                            if l == 0:
                                for kt in range(N // 128):
                                    nc.tensor.matmul(
                                        out=ps[:],
                                        lhsT=ctb[:, kt, kk * CHW:(kk + 1) * CHW],
                                        rhs=t1sb[:, kt, :],
                                        start=(kt == 0), stop=False)
                            else:
                                xb = xbufs[k % 3]
                                ib = (s * SEGSLOTS + kk * ACAP) // 16
                                crega = nc.gpsimd.value_load(cnts_t[0:1, k:k + 1])
                                nc.gpsimd.dma_gather(
                                    xb[:], gsrc[:], idx_t[:, ib:ib + 64],
                                    1024, crega, F2)
                                svv = st[:].rearrange("p (d b) -> p d b", b=nbat)
                                for bq in range(8):
                                    nc.tensor.matmul(
                                        out=ps[:],
                                        lhsT=svv[:, :, kk * 8 + bq],
                                        rhs=xb[:, bq, :],
                                        start=(bq == 0), stop=False)
                                nc.tensor.matmul(
                                    out=ps[:], lhsT=svv[:, :, 64 + kk],
                                    rhs=tb[:, kk, :], start=False, stop=False)
                            ochunk = own[(k % 2) * 64:(k % 2) * 64 + 64, k // 2, :]
                            nc.tensor.matmul(out=ps[:],
                                             lhsT=i64_t[(k % 2) * 64:(k % 2) * 64 + 64, :],
                                             rhs=ochunk, start=False, stop=True)
                            msb = gp.tile([64, F2], dt.bfloat16, tag="msb", bufs=3)
                            nc.scalar.activation(msb[:], ps[:], AF.Copy)
                            for j in range(2):
                                tp = pp.tile([128, 64], dt.bfloat16, tag="tp", bufs=2)
                                nc.tensor.transpose(
                                    tp[:], msb[:, j * 128:(j + 1) * 128], i64_t[0:64, :])
                                dst_col = slice(k * 64, (k + 1) * 64)
                                if l == 0:
                                    nc.scalar.activation(
                                        hT_cur[:, j, dst_col], tp[:], AF.Gelu,
                                        bias=b1_t[:, j:j + 1])
                                else:
                                    nc.vector.tensor_copy(
                                        out=rhsT[:, j, dst_col], in_=tp[:])
                    if l > 0:
                        # node matmul with W{l+1} + gelu
                        wt, bt = (w2t_t, b2_t) if l == 1 else (w3t_t, b3_t)
                        fouth = 2 if l == 1 else 1
                        for jo in range(fouth):
                            for m in range(NPC // 512):
                                ps2 = pp.tile([128, 512], dt.float32, tag="nm", bufs=2)
                                for ji in range(2):
                                    if l == 1:
                                        wslice = wt[:, (2 * ji + jo) * 128:(2 * ji + jo + 1) * 128]
                                    else:
                                        wslice = wt[:, ji * 128:(ji + 1) * 128]
                                    nc.tensor.matmul(
                                        out=ps2[:], lhsT=wslice,
                                        rhs=rhsT[:, ji, m * 512:(m + 1) * 512],
                                        start=(ji == 0), stop=(ji == 1))
                                nc.scalar.activation(
                                    hT_cur[:, jo, m * 512:(m + 1) * 512],
                                    ps2[:], AF.Gelu, bias=bt[:, jo:jo + 1])
                    if l < 2:
                        # transpose hT -> node-major, then AllGather
                        hon = hown[(l + 1) % 2]
                        for t in range(NPC // 128):
                            for j in range(2):
                                tp2 = pp.tile([128, 128], dt.bfloat16, tag="tp2", bufs=2)
                                nc.tensor.transpose(
                                    tp2[:], hT_cur[:, j, t * 128:(t + 1) * 128],
                                    i128_t[:])
                                nc.vector.tensor_copy(
                                    out=hon[:, t, j * 128:(j + 1) * 128],
                                    in_=tp2[:])
                        agin = dram.tile([NPC, F2], dt.bfloat16, tag="agin")
                        agout = dram.tile([B * N, F2], dt.bfloat16, tag="agout")
                        nc.sync.dma_start(
                            out=agin.rearrange("(t p) f -> p t f", p=128),
                            in_=hon[:])
                        if variant == "sim1":
                            # local stand-in for AllGather: same HBM write
                            # volume on the receive side
                            for cc in range(NCORES):
                                nc.sync.dma_start(
                                    out=agout[cc * NPC:(cc + 1) * NPC, :],
                                    in_=agin[:])
                        else:
                            nc.gpsimd.collective_compute(
                                "AllGather", OP.bypass,
                                replica_groups=[list(range(NCORES))],
                                ins=[agin.opt()], outs=[agout.opt()])
                        gather_srcs.append(agout)

            # ---------------- attention + layernorm ----------------
            h3T = hT[2 % 2]  # [128, 2, NPC]; only [:, 0, :] is meaningful
            with tc.tile_pool(name="att_ps", bufs=1, space="PSUM") as ap_, \
                 tc.tile_pool(name="att_sb", bufs=1) as asb:
                kT = asb.tile([128, NPC], dt.bfloat16, tag="kT")
                vnm = asb.tile([128, NPC // 128, 128], dt.bfloat16, tag="vnm")
                for m in range(NPC // 512):
                    psk = ap_.tile([128, 512], dt.float32, tag="pbig", bufs=2)
                    nc.tensor.matmul(out=psk[:], lhsT=wk_t[:],
                                     rhs=h3T[:, 0, m * 512:(m + 1) * 512])
                    nc.vector.tensor_scalar(
                        out=kT[:, m * 512:(m + 1) * 512], in0=psk[:],
                        scalar1=bk_t[:], scalar2=None, op0=OP.add)
                for t in range(NPC // 128):
                    psv = ap_.tile([128, 128], dt.float32, tag="pbig", bufs=2)
                    nc.tensor.matmul(out=psv[:],
                                     lhsT=h3T[:, 0, t * 128:(t + 1) * 128],
                                     rhs=wv_t[:])
                    nc.vector.tensor_copy(out=vnm[:, t, :], in_=psv[:])
                ctx_all = asb.tile([128, 4], dt.bfloat16, tag="ctx_all")
                for g in range(GPC):
                    ssc = ap_.tile([4, 1024], dt.float32, tag="pbig", bufs=2)
                    for hh in range(2):
                        nc.tensor.matmul(
                            out=ssc[:, hh * 512:(hh + 1) * 512], lhsT=qblk_t[:],
                            rhs=kT[:, g * 1024 + hh * 512: g * 1024 + (hh + 1) * 512])
                    expt = asb.tile([4, 1024], dt.bfloat16, tag="expt")
                    sums = asb.tile([4, 1], dt.float32, tag="sums")
                    nc.scalar.activation(expt[:], ssc[:], AF.Exp,
                                         accum_out=sums[:])
                    nc.vector.tensor_add(out=sums[:], in0=sums[:], in1=ecls_t[:])
                    psr = ap_.tile([128, 1], dt.float32, tag="ptiny", bufs=2)
                    nc.tensor.matmul(out=psr[:], lhsT=r4_t[:], rhs=sums[:])
                    rbc = asb.tile([128, 1], dt.float32, tag="rbc")
                    nc.vector.reciprocal(rbc[:], psr[:])
                    psctx = ap_.tile([128, 4], dt.float32, tag="psctx", bufs=1)
                    for t in range(8):
                        pst = ap_.tile([128, 4], dt.bfloat16, tag="ptiny", bufs=2)
                        nc.tensor.transpose(
                            pst[:], expt[:, t * 128:(t + 1) * 128], i4_t[:])
                        ets = asb.tile([128, 4], dt.bfloat16, tag="ets")
                        nc.vector.tensor_copy(out=ets[:], in_=pst[:])
                        nc.tensor.matmul(out=psctx[:],
                                         lhsT=vnm[:, g * 8 + t, :], rhs=ets[:],
                                         start=(t == 0), stop=False)
                    nc.tensor.matmul(out=psctx[:], lhsT=vc4_t[:], rhs=e4_t[:],
                                     start=False, stop=True)
                    tmp4 = asb.tile([128, 4], dt.float32, tag="tmp4")
                    nc.vector.tensor_tensor(out=tmp4[:], in0=psctx[:],
                                            in1=msel_t[:], op=OP.mult)
                    ctxv = asb.tile([128, 1], dt.float32, tag="ctxv")
                    nc.vector.reduce_sum(out=ctxv[:], in_=tmp4[:],
                                         axis=mybir.AxisListType.X)
                    nc.vector.tensor_scalar(out=ctxv[:], in0=ctxv[:],
                                            scalar1=rbc[:], scalar2=bv_t[:],
                                            op0=OP.mult, op1=OP.add)
                    nc.vector.tensor_copy(out=ctx_all[:, g:g + 1], in_=ctxv[:])
                psao = ap_.tile([128, 4], dt.float32, tag="ptiny", bufs=2)
                nc.tensor.matmul(out=psao[:], lhsT=wo_t[:], rhs=ctx_all[:])
                ysb = asb.tile([128, 4], dt.float32, tag="ysb")
                nc.vector.tensor_scalar(out=ysb[:], in0=psao[:],
                                        scalar1=ynb_t[:], scalar2=None,
                                        op0=OP.add)
                psy = ap_.tile([4, 128], dt.float32, tag="ptiny", bufs=2)
                nc.tensor.matmul(out=psy[:], lhsT=ysb[:], rhs=i128f_t[:],
                                 is_transpose=True)
                yt = asb.tile([4, 128], dt.float32, tag="yt")
                nc.vector.tensor_copy(out=yt[:], in_=psy[:])
                mn = asb.tile([4, 1], dt.float32, tag="mn")
                nc.vector.reduce_sum(out=mn[:], in_=yt[:],
                                     axis=mybir.AxisListType.X)
                nc.vector.tensor_scalar(out=mn[:], in0=mn[:],
                                        scalar1=1.0 / D, scalar2=None,
                                        op0=OP.mult)
                xc = asb.tile([4, 128], dt.float32, tag="xc")
                nc.vector.tensor_scalar(out=xc[:], in0=yt[:], scalar1=mn[:],
                                        scalar2=None, op0=OP.subtract)
                sq = asb.tile([4, 128], dt.float32, tag="sq")
                ss = asb.tile([4, 1], dt.float32, tag="ss")
                nc.scalar.activation(sq[:], xc[:], AF.Square, accum_out=ss[:])
                sd = asb.tile([4, 1], dt.float32, tag="sd")
                nc.scalar.activation(sd[:], ss[:], AF.Sqrt, bias=eps_t[:],
                                     scale=1.0 / D)
                rr = asb.tile([4, 1], dt.float32, tag="rr")
                nc.vector.reciprocal(rr[:], sd[:])
                yn = asb.tile([4, 128], dt.float32, tag="yn")
                nc.vector.tensor_scalar(out=yn[:], in0=xc[:], scalar1=rr[:],
                                        scalar2=None, op0=OP.mult)
                nc.vector.tensor_tensor(out=yn[:], in0=yn[:], in1=lng_t[:],
                                        op=OP.mult)
                nc.vector.tensor_tensor(out=yn[:], in0=yn[:], in1=lnb_t[:],
                                        op=OP.add)
                nc.sync.dma_start(out=y_out[:], in_=yn[:])

    nc.compile()
    _prog_cache[variant] = nc
    return nc


def _wrap16(arr):
    """slot i -> [i % 16, i // 16], replicated into partitions 16..31.

    CoreSim's gather ucode reads partitions 0..15; the deployed HW ucode
    reads 16..31 -- fill both so either path sees the indices.
    """
    n = arr.shape[0]
    out = np.zeros((128, n // 16), np.int16)
    w = arr.reshape(n // 16, 16).T.astype(np.int16)
    out[0:16] = w
    out[16:32] = w
    return out


def _host_prep(inputs):
    node_ids = np.asarray(inputs["node_ids"]).astype(np.int64)
    src = np.asarray(inputs["src"]).astype(np.int64)
    dst = np.asarray(inputs["dst"]).astype(np.int64)
    pad_mask = np.asarray(inputs["pad_mask"])
    ew = np.asarray(inputs["edge_weight"]).astype(np.float64)
    embed = np.asarray(inputs["embed_table"]).astype(np.float64)
    W1 = np.asarray(inputs["W1"]).astype(np.float64)
    b1 = np.asarray(inputs["b1"]).astype(np.float32)
    W2 = np.asarray(inputs["W2"]).astype(np.float32)
    b2 = np.asarray(inputs["b2"]).astype(np.float32)
    W3 = np.asarray(inputs["W3"]).astype(np.float32)
    b3 = np.asarray(inputs["b3"]).astype(np.float32)
    ipw = np.asarray(inputs["in_proj_w"]).astype(np.float64)
    ipb = np.asarray(inputs["in_proj_b"]).astype(np.float64)
    ow = np.asarray(inputs["out_w"]).astype(np.float32)
    ob = np.asarray(inputs["out_b"]).astype(np.float32)
    cls = np.asarray(inputs["cls_embedding"]).astype(np.float64).reshape(D)
    ln_g = np.asarray(inputs["ln_g"]).astype(np.float32)
    ln_b = np.asarray(inputs["ln_b"]).astype(np.float32)

    assert not pad_mask.any(), "kernel compiled for all-False pad_mask"

    # ---- shared (replicated) constants ----
    T1 = (embed @ W1).astype(BF16)                       # [1024, 256]
    Wq, Wk, Wv = ipw[:, :D], ipw[:, D:2 * D], ipw[:, 2 * D:]
    bq, bk_, bv_ = ipb[:D], ipb[D:2 * D], ipb[2 * D:]
    q_cls = (cls @ Wq + bq) / np.sqrt(HD)                # [128]
    k_cls = cls @ Wk + bk_
    v_cls = cls @ Wv + bv_
    s_cls = np.array([q_cls[h * HD:(h + 1) * HD] @ k_cls[h * HD:(h + 1) * HD]
                      for h in range(H)])                # [4]
    e_cls = np.exp(s_cls)
    qblk = np.zeros((128, 4), np.float32)
    for h in range(H):
        qblk[h * HD:(h + 1) * HD, h] = q_cls[h * HD:(h + 1) * HD]
    vc4 = np.zeros((4, 128), np.float32)
    for h in range(H):
        vc4[h, h * HD:(h + 1) * HD] = v_cls[h * HD:(h + 1) * HD]
    e4 = np.diag(e_cls).astype(np.float32)
    msel = np.zeros((128, 4), np.float32)
    for h in range(H):
        msel[h * HD:(h + 1) * HD, h] = 1.0
    r4 = np.zeros((4, 128), np.float32)
    for h in range(H):
        r4[h, h * HD:(h + 1) * HD] = 1.0
    w2tiles = np.concatenate(
        [W2[ji * 128:(ji + 1) * 128, jo * 128:(jo + 1) * 128]
         for ji in range(2) for jo in range(2)], axis=1)  # [128, 512]
    w3tiles = np.concatenate(
        [W3[ji * 128:(ji + 1) * 128, :] for ji in range(2)], axis=1)
    shared = {
        "t1": T1,
        "iota_bd": np.tile(np.repeat(np.arange(CHW, dtype=np.float32), NBAT),
                           (128, 1)).astype(BF16),
        "i64": np.vstack([np.eye(64, dtype=np.float32)] * 2).astype(BF16),
        "i128": np.eye(128, dtype=np.float32).astype(BF16),
        "i128f": np.eye(128, dtype=np.float32),
        "i4": np.eye(4, dtype=np.float32).astype(BF16),
        "w2t": w2tiles.astype(BF16),
        "b2": b2.reshape(2, 128).T.copy(),
        "w3t": w3tiles.astype(BF16),
        "b3": b3.reshape(1, 128).T.copy(),
        "b1": b1.astype(np.float32).reshape(2, 128).T.copy(),
        "wk": Wk.astype(BF16),
        "bk": bk_.astype(np.float32).reshape(128, 1),
        "wv": Wv.astype(BF16),
        "bv": bv_.astype(np.float32).reshape(128, 1),
        "qblk": qblk.astype(BF16),
        "vc4": vc4.astype(BF16),
        "e4": e4.astype(BF16),
        "msel": msel,
        "r4": r4,
        "ecls": e_cls.astype(np.float32).reshape(4, 1),
        "eps": np.full((4, 1), 1e-5, np.float32),
        "ynb": (cls + ob).astype(np.float32).reshape(128, 1),
        "wo": ow.astype(BF16),
        "lng": np.tile(ln_g, (4, 1)),
        "lnb": np.tile(ln_b, (4, 1)),
    }

    # ---- per-core edge partitioning ----
    ew32 = ew.astype(np.float32)
    core_of = dst >> 12           # dst // 4096
    in_maps = []
    order_all = np.argsort(dst, kind='stable')
    dst_sorted = dst[order_all]
    core_starts = np.searchsorted(dst_sorted, np.arange(0, B * N + 1, NPC))
    chunk_starts = np.searchsorted(dst_sorted, np.arange(0, B * N + 1, CHW))
    for c in range(NCORES):
        lo, hi = core_starts[c], core_starts[c + 1]
        eidx = order_all[lo:hi]
        # slot arrays: per segment [8x1024 main | 8x128 tails]
        g_idx12 = np.full(CAP, -1, np.int64)
        sl_dst = np.full(NSEG * NBAT * 128, 100.0, np.float32)
        sl_ew = np.zeros(NSEG * NBAT * 128, np.float32)
        counts = np.zeros(NCH + NSEG, np.int32)
        base_chunk = c * NCH
        for k in range(NCH):
            a = chunk_starts[base_chunk + k] - lo
            bnd = chunk_starts[base_chunk + k + 1] - lo
            cnt = bnd - a
            assert cnt <= CHCAP, f"chunk overflow: {cnt} > {CHCAP}"
            e = eidx[a:bnd]
            s, kk = divmod(k, SEG)
            amain = min(cnt, ACAP)
            em, et = e[:amain], e[amain:]
            s0 = s * SEGSLOTS + kk * ACAP
            t0 = s * SEGSLOTS + SEG * ACAP + kk * TAIL
            g_idx12[s0:s0 + amain] = src[em]
            g_idx12[t0:t0 + cnt - amain] = src[et]
            if cnt - amain < TAIL:
                g_idx12[t0 + max(cnt - amain, 1):t0 + TAIL] = -1
                if cnt - amain == 0:
                    g_idx12[t0] = 0
            # dst_local / ew by batch: main batches kk*8+bq, tail batch 64+kk
            dl = (dst[e] - (c * NPC + k * CHW)).astype(np.float32)
            we = ew32[e]
            bmain0 = s * NBAT * 128 + (kk * 8) * 128
            sl_dst[bmain0:bmain0 + amain] = dl[:amain]
            sl_ew[bmain0:bmain0 + amain] = we[:amain]
            bt0 = s * NBAT * 128 + (64 + kk) * 128
            sl_dst[bt0:bt0 + cnt - amain] = dl[amain:]
            sl_ew[bt0:bt0 + cnt - amain] = we[amain:]
            counts[k] = max(amain, 1)
            if cnt == 0:
                g_idx12[s0] = 0
        # per-segment tail-call counts: up to last real tail slot; interior
        # dummies (idx 0) count as present
        for s in range(NSEG):
            t0 = s * SEGSLOTS + SEG * ACAP
            seg_tail = g_idx12[t0:t0 + SEG * TAIL]
            nz = np.nonzero(seg_tail >= 0)[0]
            if len(nz) == 0:
                g_idx12[t0] = 0
                counts[NCH + s] = 1
            else:
                last = nz[-1]
                # interior -1s must be 0 (gathered dummies)
                interior = seg_tail[:last + 1] < 0
                idxs = np.nonzero(interior)[0]
                g_idx12[t0 + idxs] = 0
                counts[NCH + s] = last + 1
        nids_own = node_ids[c * NPC:(c + 1) * NPC]
        # layer-0 weighted count matrix C[d_local, id] = sum ew over edges
        ids_e = node_ids[src[eidx]]
        dl_e = dst[eidx] - c * NPC
        Cf = np.bincount(dl_e * N + ids_e, weights=ew[eidx],
                         minlength=NPC * N).reshape(NPC, N).astype(np.float32)
        CtT = Cf.T.astype(BF16)          # [N ids, NPC]
        ct_tiles = CtT.reshape(N // 128, 128, NPC).transpose(1, 0, 2).copy()
        m = dict(shared)
        m.update({
            "h0w1own": T1.astype(np.float32)[nids_own].astype(BF16),
            "ct": ct_tiles,
            "idx12": _wrap16(g_idx12),
            "cnts": counts.reshape(1, NCH + NSEG),
            "dstl": sl_dst.reshape(NSEG * NBAT, 128).T.astype(BF16).copy(),
            "eww": sl_ew.reshape(NSEG * NBAT, 128).T.astype(BF16).copy(),
        })
        in_maps.append(m)
    return in_maps


def kernel(**inputs):
    from concourse.bass_utils import run_bass_kernel_spmd
    nc = _build_program()
    in_maps = _host_prep(inputs)
    res = run_bass_kernel_spmd(nc, in_maps, core_ids=list(range(NCORES)))
    y = np.concatenate([res.results[c]["y"] for c in range(NCORES)], axis=0)
    return np.ascontiguousarray(y.astype(np.float32))
